# revision 7
# baseline (speedup 1.0000x reference)
"""Block-local self-attention (BlockLocalSelfAttention) on 8 TRN2 NeuronCores.

Sharding: the 32 (batch, head) slices are split 4-per-core (pure data/head
parallelism, no collectives). Each slice is t=4096, d=64, block=128: every
128-query block attends to a 3-block local window plus one global token
(key/value 0), and query 0 additionally attends to all 4096 keys.

v2 design notes (all bf16, fp32 PSUM accumulation):
  - attention_mask is all-zeros by spec, so kt carries the RAW mask row
    (no -inf on the local copy of position 0). The local copy of key 0 in
    query blocks 0/1's windows then equals the global-slot term exactly, so
    those blocks skip the rank-1 global-slot matmul and the global-query
    riders can use kt for every block (no separate raw-mask copy).
  - qt is laid out [DA, NB, 129]: col 128 of each block is a copy of q0, so
    each K-sweep matmul computes the global-query rider in the same
    instruction (no separate M=1 matmuls that force a PE tile reconfig).
  - The global-slot probs pg are consolidated into pg_flat [128, T] with the
    32 real rows replicated 4x; v0r has v0 only on partition 0. The rank-1
    global-slot matmul then runs as a full 128x128-tile op (K=32 matmuls
    measured ~+100ns each on HW from the array reconfig).
  - Inputs DMA in chunks (8 for qt/kt, 4 for v) so the first matmuls start
    ~2us into the slice instead of waiting for whole-tensor loads.
  - Output staging and the DRAM output are bf16 (host casts to fp32);
    halves the output traffic and the DVE write cost.
"""

import os
from contextlib import ExitStack

import ml_dtypes
import numpy as np

N_CORES = 8
N, H, T, D = 2, 16, 4096, 64
BLK = 128
NB = T // BLK           # 32 key/query blocks
S = (N * H) // N_CORES  # 4 slices per core
DA = D + 1              # contraction dim (ones/mask row keeps tile_size 128)
VA = D + 1              # V augmented with ones column
BW = BLK + 1            # per-block qt columns (128 queries + q0 rider)
GSZ = 2                 # key chunks per exp group
NGRP = NB // GSZ
OG = 8                  # query blocks per output staging tile
RP_BUFS = int(os.environ.get("KRPBUFS", "3"))
CX_BUFS = int(os.environ.get("KCXBUFS", "2"))
MAXR = 3                # max query blocks finalized per group
PT_BUFS = int(os.environ.get("KPTBUFS", "3"))
QT_CHUNKS = int(os.environ.get("KQTC", "8"))
KT_CHUNKS = int(os.environ.get("KKTC", "8"))
VT_CHUNKS = int(os.environ.get("KVTC", "4"))

_CACHE = {}
LAST_RESULTS = None  # BassKernelResults of the most recent run (for test.py)


def _install_ntff_shim():
    """Register an antenv.axon_hooks NTFF profile hook backed by direct
    ctypes calls into libaxon_pjrt.so, so trace=True yields a real
    neuron-profile capture in this container. No-op if unavailable."""
    import contextlib
    import ctypes
    import sys
    import types

    if "antenv.axon_hooks" in sys.modules:
        return True
    try:
        lib = ctypes.CDLL("/opt/axon/libaxon_pjrt.so")
        lib.axon_start_nrt_profile.argtypes = [
            ctypes.POINTER(ctypes.c_int64),
            ctypes.c_size_t,
        ]
        lib.axon_start_nrt_profile.restype = ctypes.c_int64
        lib.axon_stop_nrt_profile.argtypes = [ctypes.c_char_p]
        lib.axon_stop_nrt_profile.restype = ctypes.c_int64
    except Exception:
        return False

    @contextlib.contextmanager
    def _hook(output_dir, device_ids):
        import jax

        jax.devices()
        if device_ids:
            ids = (ctypes.c_int64 * len(device_ids))(*device_ids)
            rc = lib.axon_start_nrt_profile(ids, len(device_ids))
        else:
            rc = lib.axon_start_nrt_profile(None, 0)
        if rc != 0:
            raise RuntimeError(f"axon_start_nrt_profile rc={rc}")
        try:
            yield
        finally:
            lib.axon_stop_nrt_profile(str(output_dir).encode())

    mod = types.ModuleType("antenv.axon_hooks")
    mod.get_axon_ntff_profile_hook = lambda: _hook
    mod.set_axon_ntff_profile_hook = lambda h: None
    sys.modules["antenv.axon_hooks"] = mod

    from concourse import bass_utils

    bass_utils.upload_artifacts = lambda tmpdir: f"local:{tmpdir}"
    return True


def _build_program(reps=1):
    import concourse.bass as bass  # noqa: F401
    import concourse.tile as tile
    from concourse import bacc, mybir

    f32 = mybir.dt.float32
    bf16 = mybir.dt.bfloat16
    EXP = mybir.ActivationFunctionType.Exp

    nc = bacc.Bacc("TRN2", target_bir_lowering=False, debug=False)

    qt_d = nc.dram_tensor("qt", [S, DA, NB + 2, BW], bf16, kind="ExternalInput").ap()
    kt_d = nc.dram_tensor("kt", [S, DA, T], bf16, kind="ExternalInput").ap()
    k0g_d = nc.dram_tensor("k0g", [S, DA, 32], bf16, kind="ExternalInput").ap()
    v_d = nc.dram_tensor("v", [S, BLK, NB, VA], bf16, kind="ExternalInput").ap()
    v0r_d = nc.dram_tensor("v0r", [S, BLK, VA], bf16, kind="ExternalInput").ap()
    out_d = nc.dram_tensor("out", [S, T, D], bf16, kind="ExternalOutput").ap()
    o0x_d = nc.dram_tensor("o0x", [S, VA], f32, kind="ExternalOutput").ap()
    zr_d = nc.dram_tensor("zr", [96, T], bf16, kind="ExternalInput").ap()

    with tile.TileContext(nc) as tc, ExitStack() as ctx:
        io = ctx.enter_context(tc.tile_pool(name="io", bufs=2))
        rp = ctx.enter_context(tc.tile_pool(name="rp", bufs=RP_BUFS, space="PSUM"))
        cxp = ctx.enter_context(tc.tile_pool(name="cxp", bufs=CX_BUFS, space="PSUM"))
        ptp = ctx.enter_context(tc.tile_pool(name="ptp", bufs=PT_BUFS))
        pgp = ctx.enter_context(tc.tile_pool(name="pgp", bufs=2))
        p0p = ctx.enter_context(tc.tile_pool(name="p0p", bufs=3))
        outp = ctx.enter_context(tc.tile_pool(name="outp", bufs=3))
        recp = ctx.enter_context(tc.tile_pool(name="recp", bufs=3))

        def load_slice_inputs(s):
            # DMA issue costs ~500ns each on the issuing sequencer, so slice
            # 0 (not hidden behind prior compute) uses fine chunks for an
            # early start while later slices use few issues. All input-load
            # issues happen up front (see build_body) so transfers for
            # slices s+1/s+2 run as soon as their buffers free, instead of
            # trickling out of the SP stream one slice at a time.
            k0g = io.tile([DA, 32], bf16, tag="k0g", bufs=2)
            nc.sync.dma_start(out=k0g, in_=k0g_d[s])
            qt = io.tile([DA, NB + 2, BW], bf16, tag="qt", bufs=3)
            nqc = 2
            bnds = [(NB + 2) * c // nqc for c in range(nqc + 1)]
            for c in range(nqc):
                nc.sync.dma_start(
                    out=qt[:, bnds[c] : bnds[c + 1], :],
                    in_=qt_d[s, :, bnds[c] : bnds[c + 1], :],
                )
            kt = io.tile([DA, T], bf16, tag="kt", bufs=3)
            nkc = 2
            kc = T // nkc
            for c in range(nkc):
                nc.sync.dma_start(
                    out=kt[:, c * kc : (c + 1) * kc],
                    in_=kt_d[s, :, c * kc : (c + 1) * kc],
                )
            vt = io.tile([BLK, NB, VA], bf16, tag="v", bufs=3)
            nvc = 1
            vc = NB // nvc
            for c in range(nvc):
                nc.sync.dma_start(
                    out=vt[:, c * vc : (c + 1) * vc, :],
                    in_=v_d[s, :, c * vc : (c + 1) * vc, :],
                )
            v0r = io.tile([BLK, VA], bf16, tag="v0r", bufs=2)
            nc.sync.dma_start(out=v0r, in_=v0r_d[s])
            return qt, kt, vt, k0g, v0r

        def build_slice(s, pg_flat_bufs, ins, prefetch):
            qt, kt, vt, k0g, v0r = ins

            # ---- global-token-slot scores for every query: pg = exp(q . k0) ----
            # 8 matmuls [1, 512] spread over partitions {0,32,64} and the banks
            # of transient score-pool tiles. The rhs AP strides over qt's
            # per-block rider columns so only real query columns stream.
            spt = 3 * GSZ  # slots per r-pool tile
            n_sg_tiles = -(-8 // spt)
            sg_tiles, pg_tiles = [], []
            for t in range(n_sg_tiles):
                sgt = rp.tile([BLK, GSZ, 512], f32, tag="r", bufs=RP_BUFS,
                              name=f"sg_{s}_{t}")
                sg_tiles.append(sgt)
            for kk in range(8):
                t, r = divmod(kk, spt)
                bank, jj = divmod(r, 3)
                qrhs = bass.AP(
                    qt.tensor,
                    qt.offset + (4 * kk + 1) * BW,
                    [list(qt.ap)[0], [BW, 4], [1, BLK]],
                )
                nc.tensor.matmul(
                    out=sg_tiles[t][32 * jj : 32 * jj + 32, bank, :],
                    lhsT=k0g,
                    rhs=qrhs,
                    start=True,
                    stop=True,
                    skip_group_check=True,
                )
            for t in range(n_sg_tiles):
                sgt = sg_tiles[t]
                pgt = pgp.tile([BLK, GSZ, 512], bf16, tag="pg", bufs=2,
                               name=f"pg_{s}_{t}")
                nslots = min(8 - t * spt, spt)
                full_banks, partial = divmod(nslots, 3)
                if full_banks:
                    nc.scalar.activation(
                        out=pgt[0:96, 0:full_banks, :],
                        in_=sgt[0:96, 0:full_banks, :],
                        func=EXP,
                    )
                if partial:
                    nc.scalar.activation(
                        out=pgt[0 : 32 * partial, full_banks, :],
                        in_=sgt[0 : 32 * partial, full_banks, :],
                        func=EXP,
                    )
                pg_tiles.append(pgt)

            # Consolidate pg into pg_flat [128, T], replicating the 32 real
            # rows at partition offsets {0,32,64,96}. The rank-1 global-slot
            # matmuls then run with K=128 (full PE tile, no reconfig); v0r is
            # zero on partitions 1..127 so the replicas contribute nothing.
            pg_flat = pg_flat_bufs[s % 2]
            for t in range(n_sg_tiles):
                for jj in range(3):
                    slots = [kk for kk in range(8)
                             if kk // spt == t and (kk % spt) % 3 == jj]
                    if not slots:
                        continue
                    banks = [(kk % spt) // 3 for kk in slots]
                    assert banks == list(range(banks[0], banks[0] + len(banks)))
                    src_ap = pg_tiles[t][32 * jj : 32 * jj + 32,
                                         banks[0] : banks[0] + len(banks), :]
                    if len(slots) > 1:
                        dst = bass.AP(
                            pg_flat.tensor,
                            pg_flat.offset + 512 * slots[0],
                            [[list(pg_flat.ap)[0][0], 32],
                             [512 * 3, len(slots)], [1, 512]],
                        )
                    else:
                        dst = pg_flat[0:32,
                                      512 * slots[0] : 512 * slots[0] + 512]
                    nc.scalar.dma_start(out=dst, in_=src_ap)

            # Issue slice s+2's input DMAs here: on the SP stream they sit
            # after this slice's consolidation issues (whose sg-exp waits
            # resolve early in the slice) and BEFORE this slice's flush
            # issues (which resolve late), so prefetch transfers always run
            # a full slice ahead and never queue behind end-of-slice waits.
            prefetch()

            # ---- K-ordered sweep ----
            pts = {}     # group -> PT tile [128, gsz, 387]
            stages = {}  # out-group -> staging tile [128, OG, D]

            def do_pv(b, ctx_ap):
                """Accumulate ctx for query block b into ctx_ap [128, VA]."""
                chunks = [x for x in (b - 1, b, b + 1) if 0 <= x < NB]
                has_pvg = b >= 2
                for i, bb in enumerate(chunks):
                    pt_t = pts[bb // GSZ]
                    co = (b - bb + 1) * BW
                    nc.tensor.matmul(
                        out=ctx_ap,
                        lhsT=pt_t[:, bb % GSZ, co : co + BLK],
                        rhs=vt[:, bb, :],
                        start=(i == 0),
                        stop=(not has_pvg and i == len(chunks) - 1),
                        skip_group_check=True,
                    )
                if has_pvg:
                    nc.tensor.matmul(
                        out=ctx_ap,
                        lhsT=pg_flat[:, b * BLK : (b + 1) * BLK],
                        rhs=v0r,
                        start=False,
                        stop=True,
                        skip_group_check=True,
                    )

            def flush_stage(gi):
                # All input loads are issued up front, so the SP sequencer is
                # idle mid-run and dependent DMAs can use its hardware DGE
                # (the GpSimd SWDGE path costs ~1us per DMA plus a ~9us
                # end-of-program drain). Split the last slice's flushes so
                # the final transfer drains across several rings.
                nsp = 2 if s == S - 1 else 1
                half = OG // nsp
                for hh in range(nsp):
                    dst = out_d[
                        s,
                        (gi * OG + hh * half) * BLK : (gi * OG + (hh + 1) * half) * BLK,
                        :,
                    ].rearrange("(j p) d -> p j d", p=BLK)
                    nc.sync.dma_start(
                        out=dst, in_=stages[gi][:, hh * half : (hh + 1) * half, :]
                    )

            def finalize(blocks, ctxg):
                nb2 = len(blocks)
                rec = recp.tile([BLK, MAXR, 1], f32, tag="rec", bufs=3)
                nc.vector.reciprocal(
                    out=rec[:, 0:nb2, :], in_=ctxg[:, 0:nb2, D : D + 1]
                )
                for j, b in enumerate(blocks):
                    gi = b // OG
                    if gi not in stages:
                        stages[gi] = outp.tile(
                            [BLK, OG, D], bf16, tag="out", bufs=3,
                            name=f"stage_{s}_{gi}",
                        )
                    nc.vector.tensor_scalar_mul(
                        out=stages[gi][:, b % OG, :],
                        in0=ctxg[:, j, 0:D],
                        scalar1=rec[:, j, :],
                    )
                    if b % OG == OG - 1:
                        flush_stage(gi)

            p0 = p0p.tile([BLK, NB], bf16, tag="p0", bufs=3)
            prev_blocks, prev_ctx = [], None
            for g in range(NGRP):
                chunks = list(range(g * GSZ, (g + 1) * GSZ))
                r_t = rp.tile([BLK, GSZ, 512], f32, tag="r", bufs=RP_BUFS)
                for i, bb in enumerate(chunks):
                    # qt carries dummy copies of block 0 at aug indices 0 and
                    # NB+1, so every window matmul is full-width: dummy-block
                    # scores are finite, land in pt columns no PV reads, and
                    # their rider columns still hold the real q0 scores.
                    nc.tensor.matmul(
                        out=r_t[:, i, 0 : 3 * BW],
                        lhsT=kt[:, bb * BLK : (bb + 1) * BLK],
                        rhs=qt[:, bb : bb + 3, :],
                        start=True,
                        stop=True,
                        skip_group_check=True,
                    )
                # exp straight out of PSUM (includes the rider columns)
                pt_t = ptp.tile([BLK, GSZ, 3 * BW], bf16, tag="pt", bufs=PT_BUFS)
                nc.scalar.activation(
                    out=pt_t[:, 0:GSZ, :], in_=r_t[:, 0:GSZ, 0 : 3 * BW], func=EXP
                )
                pts[g] = pt_t
                # stash the exp'd global-query rider columns into p0
                # (every block's window carries an identical copy at
                # col j*BW+128; j=0 is always valid thanks to the dummies).
                nc.gpsimd.tensor_copy(
                    out=p0[:, chunks[0] : chunks[0] + GSZ].unsqueeze(-1),
                    in_=pt_t[:, 0:GSZ, BLK : BLK + 1],
                )
                # PV + normalize for blocks whose windows completed last group
                if prev_blocks:
                    finalize(prev_blocks, prev_ctx)
                ready = [b for b in range(chunks[0] - 1, chunks[-1]) if b >= 0]
                if g == NGRP - 1:
                    ready.append(NB - 1)
                assert len(ready) <= MAXR
                ctxg = cxp.tile(
                    [BLK, MAXR, VA], f32, tag="ctx", bufs=CX_BUFS,
                    name=f"ctx_{s}_{g}",
                )
                for j, b in enumerate(ready):
                    do_pv(b, ctxg[:, j, :])
                prev_blocks, prev_ctx = ready, ctxg
            finalize(prev_blocks, prev_ctx)

            # ---- global query (row 0): full softmax over all 4096 keys.
            # V-stationary accumulation: out is ctx0 TRANSPOSED [65, 1]
            # (d rows + denominator at row 64), streaming one p0 column per
            # key block. Written to DRAM unnormalized; the host divides.
            o0 = rp.tile([BLK, GSZ, 512], f32, tag="r", bufs=RP_BUFS)
            for bb in range(NB):
                nc.tensor.matmul(
                    out=o0[0:VA, 0, 0:1],
                    lhsT=vt[:, bb, :],
                    rhs=p0[:, bb : bb + 1],
                    start=(bb == 0),
                    stop=(bb == NB - 1),
                    skip_group_check=True,
                )
            o0s = recp.tile([BLK, MAXR, 1], f32, tag="rec", bufs=3)
            nc.vector.tensor_copy(out=o0s[0:VA, 0, :], in_=o0[0:VA, 0, 0:1])
            nc.sync.dma_start(out=o0x_d[s], in_=o0s[0:VA, 0, 0:1])

        def build_body():
            # Persistent global-slot prob tiles, double-buffered across
            # slices. PVg contracts over K=128 but only partition 0 of v0r
            # is nonzero, so rows 32..127 just need to be finite: zero them
            # once up front (hidden behind the initial input DMAs).
            pg_flat_bufs = []
            for i in range(2):
                pgf = pgp.tile([BLK, T], bf16, tag=f"pgf{i}", bufs=1,
                               name=f"pgf{i}")
                # init the replica rows from a DRAM zeros tensor: costs no
                # engine time (runs on an ACT-issued DMA ring during the
                # initial input loads).
                nc.scalar.dma_start(out=pgf[32:64, :], in_=zr_d[0:32])
                nc.scalar.dma_start(out=pgf[64:128, :], in_=zr_d[32:96])
                pg_flat_bufs.append(pgf)
            ins = [None] * S
            ins[0] = load_slice_inputs(0)
            ins[1] = load_slice_inputs(1)

            def make_prefetch(s2):
                def pf():
                    if s2 < S and ins[s2] is None:
                        ins[s2] = load_slice_inputs(s2)
                return pf

            for s in range(S):
                build_slice(s, pg_flat_bufs, ins[s], make_prefetch(s + 2))

        if reps > 1:
            with tc.For_i(0, reps, 1):
                build_body()
        else:
            build_body()

    nc.compile()
    return nc


def _prep_core_inputs(q, k, v, mask, core):
    bf = ml_dtypes.bfloat16
    scale = np.float32(1.0 / np.sqrt(D))
    qt = np.empty((S, DA, NB + 2, BW), np.float32)
    kt = np.empty((S, DA, T), np.float32)
    k0g = np.zeros((S, DA, 32), np.float32)
    vt = np.empty((S, BLK, NB, VA), np.float32)
    v0r = np.zeros((S, BLK, VA), np.float32)
    for s in range(S):
        g = core * S + s
        n, h = divmod(g, H)
        Q, K, V = q[n, h], k[n, h], v[n, h]          # [T, D]
        m = np.asarray(mask[n, 0, 0], np.float32)    # [T]
        qs = Q.T * scale                             # [D, T]
        qt[s, :D, 1 : NB + 1, 0:BLK] = qs.reshape(D, NB, BLK)
        qt[s, D, :, 0:BLK] = 1.0
        qt[s, :D, :, BLK] = qs[:, 0:1]               # q0 rider per block
        qt[s, D, :, BLK] = 1.0
        qt[s, :D, 0, 0:BLK] = qs[:, 0:BLK]           # dummy = copy of block 0
        qt[s, :D, NB + 1, 0:BLK] = qs[:, 0:BLK]
        kt[s, :D] = K.T
        kt[s, D] = m                                 # raw mask (all zeros)
        k0g[s, :D, 0] = K[0]                         # cols 1..31 stay zero
        k0g[s, D, 0] = m[0]                          # global slot mask
        va = np.concatenate([V, np.ones((T, 1), np.float32)], axis=1)
        vt[s] = va.reshape(NB, BLK, VA).transpose(1, 0, 2)
        v0r[s] = 0.0
        v0r[s, 0] = va[0]                            # v0 on partition 0 only
    return {
        "zr": np.zeros((96, T), bf),
        "qt": qt.astype(bf),
        "kt": kt.astype(bf),
        "k0g": k0g.astype(bf),
        "v": vt.astype(bf),
        "v0r": v0r.astype(bf),
    }


def kernel(query_layer, key_layer, value_layer, attention_mask):
    global LAST_RESULTS
    from concourse.bass_utils import run_bass_kernel_spmd

    q = np.ascontiguousarray(np.asarray(query_layer, dtype=np.float32))
    k = np.ascontiguousarray(np.asarray(key_layer, dtype=np.float32))
    v = np.ascontiguousarray(np.asarray(value_layer, dtype=np.float32))
    mask = np.asarray(attention_mask, dtype=np.float32)

    if "nc" not in _CACHE:
        _CACHE["nc"] = _build_program()
    nc = _CACHE["nc"]

    in_maps = [_prep_core_inputs(q, k, v, mask, c) for c in range(N_CORES)]
    trace = bool(int(os.environ.get("KERNEL_TRACE", "0")))
    if trace:
        trace = _install_ntff_shim()
    res = run_bass_kernel_spmd(nc, in_maps, list(range(N_CORES)), trace=trace)
    LAST_RESULTS = res

    out = np.empty((N, H, T, D), np.float32)
    for c in range(N_CORES):
        core_out = np.asarray(res.results[c]["out"], np.float32)  # [S, T, D]
        o0x = np.asarray(res.results[c]["o0x"], np.float32)       # [S, VA]
        for s in range(S):
            n, h = divmod(c * S + s, H)
            out[n, h] = core_out[s]
            out[n, h, 0, :] = o0x[s, :D] / o0x[s, D]
    return out


# revision 9
# speedup vs baseline: 1.0811x; 1.0811x over previous
"""Block-local self-attention (BlockLocalSelfAttention) on 8 TRN2 NeuronCores.

Sharding: the 32 (batch, head) slices are split 4-per-core (pure data/head
parallelism, no collectives). Each slice is t=4096, d=64, block=128: every
128-query block attends to a 3-block local window plus one global token
(key/value 0), and query 0 additionally attends to all 4096 keys.

v2 design notes (all bf16, fp32 PSUM accumulation):
  - attention_mask is all-zeros by spec, so kt carries the RAW mask row
    (no -inf on the local copy of position 0). The local copy of key 0 in
    query blocks 0/1's windows then equals the global-slot term exactly, so
    those blocks skip the rank-1 global-slot matmul and the global-query
    riders can use kt for every block (no separate raw-mask copy).
  - qt is laid out [DA, NB, 129]: col 128 of each block is a copy of q0, so
    each K-sweep matmul computes the global-query rider in the same
    instruction (no separate M=1 matmuls that force a PE tile reconfig).
  - The global-slot probs pg are consolidated into pg_flat [128, T] with the
    32 real rows replicated 4x; v0r has v0 only on partition 0. The rank-1
    global-slot matmul then runs as a full 128x128-tile op (K=32 matmuls
    measured ~+100ns each on HW from the array reconfig).
  - DMA issue costs ~500-800ns on the issuing sequencer, so DMAs are
    rationed and scheduled: input loads (few chunks each) and output
    flushes on the SP sequencer with slice s+2's inputs issued mid-slice-s
    (after the consolidation issues, before the flush issues) so prefetch
    never queues behind end-of-slice semaphore waits; global-slot
    consolidation also on SP; pg_flat replica-row init via a DMA from a
    DRAM zeros tensor. GpSimd only runs the small p0 stash copies (its
    SWDGE DMA path costs ~1us per DMA plus a ~9us end-of-program drain).
  - The global query (row 0) is accumulated V-stationary into a
    transposed [65, 1] PSUM column (one p0 column streamed per key
    block), written out unnormalized; the host divides by the
    denominator. Output staging and the DRAM output are bf16 (host casts
    to fp32); halves the output traffic and the DVE write cost.
"""

import os
from contextlib import ExitStack

import ml_dtypes
import numpy as np

N_CORES = 8
N, H, T, D = 2, 16, 4096, 64
BLK = 128
NB = T // BLK           # 32 key/query blocks
S = (N * H) // N_CORES  # 4 slices per core
DA = D + 1              # contraction dim (ones/mask row keeps tile_size 128)
VA = D + 1              # V augmented with ones column
BW = BLK + 1            # per-block qt columns (128 queries + q0 rider)
GSZ = 2                 # key chunks per exp group
NGRP = NB // GSZ
OG = 8                  # query blocks per output staging tile
RP_BUFS = int(os.environ.get("KRPBUFS", "3"))
CX_BUFS = int(os.environ.get("KCXBUFS", "2"))
MAXR = 3                # max query blocks finalized per group
PT_BUFS = int(os.environ.get("KPTBUFS", "3"))

_CACHE = {}
LAST_RESULTS = None  # BassKernelResults of the most recent run (for test.py)


def _install_ntff_shim():
    """Register an antenv.axon_hooks NTFF profile hook backed by direct
    ctypes calls into libaxon_pjrt.so, so trace=True yields a real
    neuron-profile capture in this container. No-op if unavailable."""
    import contextlib
    import ctypes
    import sys
    import types

    if "antenv.axon_hooks" in sys.modules:
        return True
    try:
        lib = ctypes.CDLL("/opt/axon/libaxon_pjrt.so")
        lib.axon_start_nrt_profile.argtypes = [
            ctypes.POINTER(ctypes.c_int64),
            ctypes.c_size_t,
        ]
        lib.axon_start_nrt_profile.restype = ctypes.c_int64
        lib.axon_stop_nrt_profile.argtypes = [ctypes.c_char_p]
        lib.axon_stop_nrt_profile.restype = ctypes.c_int64
    except Exception:
        return False

    @contextlib.contextmanager
    def _hook(output_dir, device_ids):
        import jax

        jax.devices()
        if device_ids:
            ids = (ctypes.c_int64 * len(device_ids))(*device_ids)
            rc = lib.axon_start_nrt_profile(ids, len(device_ids))
        else:
            rc = lib.axon_start_nrt_profile(None, 0)
        if rc != 0:
            raise RuntimeError(f"axon_start_nrt_profile rc={rc}")
        try:
            yield
        finally:
            lib.axon_stop_nrt_profile(str(output_dir).encode())

    mod = types.ModuleType("antenv.axon_hooks")
    mod.get_axon_ntff_profile_hook = lambda: _hook
    mod.set_axon_ntff_profile_hook = lambda h: None
    sys.modules["antenv.axon_hooks"] = mod

    from concourse import bass_utils

    bass_utils.upload_artifacts = lambda tmpdir: f"local:{tmpdir}"
    return True


def _build_program(reps=1):
    import concourse.bass as bass  # noqa: F401
    import concourse.tile as tile
    from concourse import bacc, mybir

    f32 = mybir.dt.float32
    bf16 = mybir.dt.bfloat16
    EXP = mybir.ActivationFunctionType.Exp

    nc = bacc.Bacc("TRN2", target_bir_lowering=False, debug=False)

    qt_d = nc.dram_tensor("qt", [S, DA, NB + 2, BW], bf16, kind="ExternalInput").ap()
    kt_d = nc.dram_tensor("kt", [S, DA, T], bf16, kind="ExternalInput").ap()
    k0g_d = nc.dram_tensor("k0g", [S, DA, 32], bf16, kind="ExternalInput").ap()
    v_d = nc.dram_tensor("v", [S, BLK, NB, VA], bf16, kind="ExternalInput").ap()
    v0r_d = nc.dram_tensor("v0r", [S, BLK, VA], bf16, kind="ExternalInput").ap()
    out_d = nc.dram_tensor("out", [S, T, D], bf16, kind="ExternalOutput").ap()
    o0x_d = nc.dram_tensor("o0x", [S, VA], f32, kind="ExternalOutput").ap()
    zr_d = nc.dram_tensor("zr", [96, T], bf16, kind="ExternalInput").ap()

    with tile.TileContext(nc) as tc, ExitStack() as ctx:
        io = ctx.enter_context(tc.tile_pool(name="io", bufs=2))
        rp = ctx.enter_context(tc.tile_pool(name="rp", bufs=RP_BUFS, space="PSUM"))
        cxp = ctx.enter_context(tc.tile_pool(name="cxp", bufs=CX_BUFS, space="PSUM"))
        ptp = ctx.enter_context(tc.tile_pool(name="ptp", bufs=PT_BUFS))
        pgp = ctx.enter_context(tc.tile_pool(name="pgp", bufs=2))
        p0p = ctx.enter_context(tc.tile_pool(name="p0p", bufs=3))
        outp = ctx.enter_context(tc.tile_pool(name="outp", bufs=3))
        recp = ctx.enter_context(tc.tile_pool(name="recp", bufs=3))

        def load_slice_inputs(s):
            # DMA issue costs ~500ns each on the issuing sequencer, so slice
            # 0 (not hidden behind prior compute) uses fine chunks for an
            # early start while later slices use few issues. All input-load
            # issues happen up front (see build_body) so transfers for
            # slices s+1/s+2 run as soon as their buffers free, instead of
            # trickling out of the SP stream one slice at a time.
            k0g = io.tile([DA, 32], bf16, tag="k0g", bufs=2)
            nc.sync.dma_start(out=k0g, in_=k0g_d[s])
            qt = io.tile([DA, NB + 2, BW], bf16, tag="qt", bufs=3)
            nqc = 4 if s == 0 else 2
            bnds = [(NB + 2) * c // nqc for c in range(nqc + 1)]
            for c in range(nqc):
                nc.sync.dma_start(
                    out=qt[:, bnds[c] : bnds[c + 1], :],
                    in_=qt_d[s, :, bnds[c] : bnds[c + 1], :],
                )
            kt = io.tile([DA, T], bf16, tag="kt", bufs=3)
            nkc = 4 if s == 0 else 2
            kc = T // nkc
            for c in range(nkc):
                nc.sync.dma_start(
                    out=kt[:, c * kc : (c + 1) * kc],
                    in_=kt_d[s, :, c * kc : (c + 1) * kc],
                )
            vt = io.tile([BLK, NB, VA], bf16, tag="v", bufs=3)
            nvc = 2 if s == 0 else 1
            vc = NB // nvc
            for c in range(nvc):
                nc.sync.dma_start(
                    out=vt[:, c * vc : (c + 1) * vc, :],
                    in_=v_d[s, :, c * vc : (c + 1) * vc, :],
                )
            v0r = io.tile([BLK, VA], bf16, tag="v0r", bufs=2)
            nc.sync.dma_start(out=v0r, in_=v0r_d[s])
            return qt, kt, vt, k0g, v0r

        def build_slice(s, pg_flat_bufs, ins, prefetch):
            qt, kt, vt, k0g, v0r = ins

            # ---- global-token-slot scores for every query: pg = exp(q . k0) ----
            # 8 matmuls [1, 512] spread over partitions {0,32,64} and the banks
            # of transient score-pool tiles. The rhs AP strides over qt's
            # per-block rider columns so only real query columns stream.
            spt = 3 * GSZ  # slots per r-pool tile
            n_sg_tiles = -(-8 // spt)
            sg_tiles, pg_tiles = [], []
            for t in range(n_sg_tiles):
                sgt = rp.tile([BLK, GSZ, 512], f32, tag="r", bufs=RP_BUFS,
                              name=f"sg_{s}_{t}")
                sg_tiles.append(sgt)
            for kk in range(8):
                t, r = divmod(kk, spt)
                bank, jj = divmod(r, 3)
                qrhs = bass.AP(
                    qt.tensor,
                    qt.offset + (4 * kk + 1) * BW,
                    [list(qt.ap)[0], [BW, 4], [1, BLK]],
                )
                nc.tensor.matmul(
                    out=sg_tiles[t][32 * jj : 32 * jj + 32, bank, :],
                    lhsT=k0g,
                    rhs=qrhs,
                    start=True,
                    stop=True,
                    skip_group_check=True,
                )
            for t in range(n_sg_tiles):
                sgt = sg_tiles[t]
                pgt = pgp.tile([BLK, GSZ, 512], bf16, tag="pg", bufs=2,
                               name=f"pg_{s}_{t}")
                nslots = min(8 - t * spt, spt)
                full_banks, partial = divmod(nslots, 3)
                if full_banks:
                    nc.scalar.activation(
                        out=pgt[0:96, 0:full_banks, :],
                        in_=sgt[0:96, 0:full_banks, :],
                        func=EXP,
                    )
                if partial:
                    nc.scalar.activation(
                        out=pgt[0 : 32 * partial, full_banks, :],
                        in_=sgt[0 : 32 * partial, full_banks, :],
                        func=EXP,
                    )
                pg_tiles.append(pgt)

            # Consolidate pg into pg_flat [128, T], replicating the 32 real
            # rows at partition offsets {0,32,64,96}. The rank-1 global-slot
            # matmuls then run with K=128 (full PE tile, no reconfig); v0r is
            # zero on partitions 1..127 so the replicas contribute nothing.
            pg_flat = pg_flat_bufs[s % 2]
            for t in range(n_sg_tiles):
                for jj in range(3):
                    slots = [kk for kk in range(8)
                             if kk // spt == t and (kk % spt) % 3 == jj]
                    if not slots:
                        continue
                    banks = [(kk % spt) // 3 for kk in slots]
                    assert banks == list(range(banks[0], banks[0] + len(banks)))
                    src_ap = pg_tiles[t][32 * jj : 32 * jj + 32,
                                         banks[0] : banks[0] + len(banks), :]
                    if len(slots) > 1:
                        dst = bass.AP(
                            pg_flat.tensor,
                            pg_flat.offset + 512 * slots[0],
                            [[list(pg_flat.ap)[0][0], 32],
                             [512 * 3, len(slots)], [1, 512]],
                        )
                    else:
                        dst = pg_flat[0:32,
                                      512 * slots[0] : 512 * slots[0] + 512]
                    nc.sync.dma_start(out=dst, in_=src_ap)

            # Issue slice s+2's input DMAs here: on the SP stream they sit
            # after this slice's consolidation issues (whose sg-exp waits
            # resolve early in the slice) and BEFORE this slice's flush
            # issues (which resolve late), so prefetch transfers always run
            # a full slice ahead and never queue behind end-of-slice waits.
            prefetch()

            # ---- K-ordered sweep ----
            pts = {}     # group -> PT tile [128, gsz, 387]
            stages = {}  # out-group -> staging tile [128, OG, D]

            def do_pv(b, ctx_ap):
                """Accumulate ctx for query block b into ctx_ap [128, VA]."""
                chunks = [x for x in (b - 1, b, b + 1) if 0 <= x < NB]
                has_pvg = b >= 2
                for i, bb in enumerate(chunks):
                    pt_t = pts[bb // GSZ]
                    co = (b - bb + 1) * BW
                    nc.tensor.matmul(
                        out=ctx_ap,
                        lhsT=pt_t[:, bb % GSZ, co : co + BLK],
                        rhs=vt[:, bb, :],
                        start=(i == 0),
                        stop=(not has_pvg and i == len(chunks) - 1),
                        skip_group_check=True,
                    )
                if has_pvg:
                    nc.tensor.matmul(
                        out=ctx_ap,
                        lhsT=pg_flat[:, b * BLK : (b + 1) * BLK],
                        rhs=v0r,
                        start=False,
                        stop=True,
                        skip_group_check=True,
                    )

            def flush_stage(gi):
                # All input loads are issued up front, so the SP sequencer is
                # idle mid-run and dependent DMAs can use its hardware DGE
                # (the GpSimd SWDGE path costs ~1us per DMA plus a ~9us
                # end-of-program drain). Split the last slice's flushes so
                # the final transfer drains across several rings.
                nsp = 2 if s == S - 1 else 1
                half = OG // nsp
                for hh in range(nsp):
                    dst = out_d[
                        s,
                        (gi * OG + hh * half) * BLK : (gi * OG + (hh + 1) * half) * BLK,
                        :,
                    ].rearrange("(j p) d -> p j d", p=BLK)
                    nc.sync.dma_start(
                        out=dst, in_=stages[gi][:, hh * half : (hh + 1) * half, :]
                    )

            def finalize(blocks, ctxg):
                nb2 = len(blocks)
                rec = recp.tile([BLK, MAXR, 1], f32, tag="rec", bufs=3)
                nc.vector.reciprocal(
                    out=rec[:, 0:nb2, :], in_=ctxg[:, 0:nb2, D : D + 1]
                )
                for j, b in enumerate(blocks):
                    gi = b // OG
                    if gi not in stages:
                        stages[gi] = outp.tile(
                            [BLK, OG, D], bf16, tag="out", bufs=3,
                            name=f"stage_{s}_{gi}",
                        )
                    nc.vector.tensor_scalar_mul(
                        out=stages[gi][:, b % OG, :],
                        in0=ctxg[:, j, 0:D],
                        scalar1=rec[:, j, :],
                    )
                    if b % OG == OG - 1:
                        flush_stage(gi)

            p0 = p0p.tile([BLK, NB], bf16, tag="p0", bufs=3)
            prev_blocks, prev_ctx = [], None
            for g in range(NGRP):
                chunks = list(range(g * GSZ, (g + 1) * GSZ))
                r_t = rp.tile([BLK, GSZ, 512], f32, tag="r", bufs=RP_BUFS)
                for i, bb in enumerate(chunks):
                    # qt carries dummy copies of block 0 at aug indices 0 and
                    # NB+1, so every window matmul is full-width: dummy-block
                    # scores are finite, land in pt columns no PV reads, and
                    # their rider columns still hold the real q0 scores.
                    nc.tensor.matmul(
                        out=r_t[:, i, 0 : 3 * BW],
                        lhsT=kt[:, bb * BLK : (bb + 1) * BLK],
                        rhs=qt[:, bb : bb + 3, :],
                        start=True,
                        stop=True,
                        skip_group_check=True,
                    )
                # exp straight out of PSUM (includes the rider columns)
                pt_t = ptp.tile([BLK, GSZ, 3 * BW], bf16, tag="pt", bufs=PT_BUFS)
                nc.scalar.activation(
                    out=pt_t[:, 0:GSZ, :], in_=r_t[:, 0:GSZ, 0 : 3 * BW], func=EXP
                )
                pts[g] = pt_t
                # stash the exp'd global-query rider columns into p0
                # (every block's window carries an identical copy at
                # col j*BW+128; j=0 is always valid thanks to the dummies).
                nc.gpsimd.tensor_copy(
                    out=p0[:, chunks[0] : chunks[0] + GSZ].unsqueeze(-1),
                    in_=pt_t[:, 0:GSZ, BLK : BLK + 1],
                )
                # PV + normalize for blocks whose windows completed last group
                if prev_blocks:
                    finalize(prev_blocks, prev_ctx)
                ready = [b for b in range(chunks[0] - 1, chunks[-1]) if b >= 0]
                if g == NGRP - 1:
                    ready.append(NB - 1)
                assert len(ready) <= MAXR
                ctxg = cxp.tile(
                    [BLK, MAXR, VA], f32, tag="ctx", bufs=CX_BUFS,
                    name=f"ctx_{s}_{g}",
                )
                for j, b in enumerate(ready):
                    do_pv(b, ctxg[:, j, :])
                prev_blocks, prev_ctx = ready, ctxg
            finalize(prev_blocks, prev_ctx)

            # ---- global query (row 0): full softmax over all 4096 keys.
            # V-stationary accumulation: out is ctx0 TRANSPOSED [65, 1]
            # (d rows + denominator at row 64), streaming one p0 column per
            # key block. Written to DRAM unnormalized; the host divides.
            o0 = rp.tile([BLK, GSZ, 512], f32, tag="r", bufs=RP_BUFS)
            for bb in range(NB):
                nc.tensor.matmul(
                    out=o0[0:VA, 0, 0:1],
                    lhsT=vt[:, bb, :],
                    rhs=p0[:, bb : bb + 1],
                    start=(bb == 0),
                    stop=(bb == NB - 1),
                    skip_group_check=True,
                )
            o0s = recp.tile([BLK, MAXR, 1], f32, tag="rec", bufs=3)
            nc.vector.tensor_copy(out=o0s[0:VA, 0, :], in_=o0[0:VA, 0, 0:1])
            nc.sync.dma_start(out=o0x_d[s], in_=o0s[0:VA, 0, 0:1])

        def build_body():
            # Persistent global-slot prob tiles, double-buffered across
            # slices. PVg contracts over K=128 but only partition 0 of v0r
            # is nonzero, so rows 32..127 just need to be finite: zero them
            # once up front (hidden behind the initial input DMAs).
            pg_flat_bufs = []
            for i in range(2):
                pgf = pgp.tile([BLK, T], bf16, tag=f"pgf{i}", bufs=1,
                               name=f"pgf{i}")
                # init the replica rows from a DRAM zeros tensor: costs no
                # engine time (runs on an ACT-issued DMA ring during the
                # initial input loads).
                nc.scalar.dma_start(out=pgf[32:64, :], in_=zr_d[0:32])
                nc.scalar.dma_start(out=pgf[64:128, :], in_=zr_d[32:96])
                pg_flat_bufs.append(pgf)
            ins = [None] * S
            ins[0] = load_slice_inputs(0)
            ins[1] = load_slice_inputs(1)

            def make_prefetch(s2):
                def pf():
                    if s2 < S and ins[s2] is None:
                        ins[s2] = load_slice_inputs(s2)
                return pf

            for s in range(S):
                build_slice(s, pg_flat_bufs, ins[s], make_prefetch(s + 2))

        if reps > 1:
            with tc.For_i(0, reps, 1):
                build_body()
        else:
            build_body()

    nc.compile()
    return nc


def _prep_core_inputs(q, k, v, mask, core):
    bf = ml_dtypes.bfloat16
    scale = np.float32(1.0 / np.sqrt(D))
    qt = np.empty((S, DA, NB + 2, BW), np.float32)
    kt = np.empty((S, DA, T), np.float32)
    k0g = np.zeros((S, DA, 32), np.float32)
    vt = np.empty((S, BLK, NB, VA), np.float32)
    v0r = np.zeros((S, BLK, VA), np.float32)
    for s in range(S):
        g = core * S + s
        n, h = divmod(g, H)
        Q, K, V = q[n, h], k[n, h], v[n, h]          # [T, D]
        m = np.asarray(mask[n, 0, 0], np.float32)    # [T]
        qs = Q.T * scale                             # [D, T]
        qt[s, :D, 1 : NB + 1, 0:BLK] = qs.reshape(D, NB, BLK)
        qt[s, D, :, 0:BLK] = 1.0
        qt[s, :D, :, BLK] = qs[:, 0:1]               # q0 rider per block
        qt[s, D, :, BLK] = 1.0
        qt[s, :D, 0, 0:BLK] = qs[:, 0:BLK]           # dummy = copy of block 0
        qt[s, :D, NB + 1, 0:BLK] = qs[:, 0:BLK]
        kt[s, :D] = K.T
        kt[s, D] = m                                 # raw mask (all zeros)
        k0g[s, :D, 0] = K[0]                         # cols 1..31 stay zero
        k0g[s, D, 0] = m[0]                          # global slot mask
        va = np.concatenate([V, np.ones((T, 1), np.float32)], axis=1)
        vt[s] = va.reshape(NB, BLK, VA).transpose(1, 0, 2)
        v0r[s] = 0.0
        v0r[s, 0] = va[0]                            # v0 on partition 0 only
    return {
        "zr": np.zeros((96, T), bf),
        "qt": qt.astype(bf),
        "kt": kt.astype(bf),
        "k0g": k0g.astype(bf),
        "v": vt.astype(bf),
        "v0r": v0r.astype(bf),
    }


def kernel(query_layer, key_layer, value_layer, attention_mask):
    global LAST_RESULTS
    from concourse.bass_utils import run_bass_kernel_spmd

    q = np.ascontiguousarray(np.asarray(query_layer, dtype=np.float32))
    k = np.ascontiguousarray(np.asarray(key_layer, dtype=np.float32))
    v = np.ascontiguousarray(np.asarray(value_layer, dtype=np.float32))
    mask = np.asarray(attention_mask, dtype=np.float32)

    if "nc" not in _CACHE:
        _CACHE["nc"] = _build_program()
    nc = _CACHE["nc"]

    in_maps = [_prep_core_inputs(q, k, v, mask, c) for c in range(N_CORES)]
    trace = bool(int(os.environ.get("KERNEL_TRACE", "0")))
    if trace:
        trace = _install_ntff_shim()
    res = run_bass_kernel_spmd(nc, in_maps, list(range(N_CORES)), trace=trace)
    LAST_RESULTS = res

    out = np.empty((N, H, T, D), np.float32)
    for c in range(N_CORES):
        core_out = np.asarray(res.results[c]["out"], np.float32)  # [S, T, D]
        o0x = np.asarray(res.results[c]["o0x"], np.float32)       # [S, VA]
        for s in range(S):
            n, h = divmod(c * S + s, H)
            out[n, h] = core_out[s]
            out[n, h, 0, :] = o0x[s, :D] / o0x[s, D]
    return out
